# revision 13
# baseline (speedup 1.0000x reference)
"""Trainium2 Bass kernel for nn_CustomGCNLayer (GCN layer, dense symmetric
adjacency from an edge list, set semantics).

Math (reference):
    h   = x @ W.T + b_lin
    A   = symmetric 0/1 adjacency from edge_index (duplicates collapse)
    Ã   = dinv[:,None] * A * dinv[None,:],  dinv = (deg+1e-6)^-0.5
    out = Ã @ h + bias

Rewritten with associativity so no h ever needs materializing:
    out = (Ã @ x) @ W.T + s b_lin^T + 1 bias^T,   s_i = Σ_j Ã[i,j]

Distribution: column shard, core k owns output rows R_k = [k*1024,(k+1)*1024).
x is replicated (it is small); there are NO collectives. Each core computes
    yT[c, i] = Σ_j x[j, c] * Ã[j, i]      (i ∈ R_k, 64 j-blocks of 128)
as 128 PSUM-accumulating bf16 matmuls (x blocks stationary, adjacency tiles
moving), then a tiny tail: yT → bf16, outT = W^T.T @ yT + [b_lin;bias].T @
[s;1], DMA outT to HBM; the host transposes/concats.

Adjacency tiles [128 j, 1024 i] (values dinv_i*dinv_j pre-scaled on host):
  - 5 of every 8 j-blocks stream as dense bf16 from HBM (host-built array),
  - 3 of every 8 are built on the fly by the Pool engine with
    gpsimd.local_scatter from per-(j-row) index/value lists,
so DMA and Pool together feed the PE fast enough to keep it at full clock
(the PE drops to half speed if it ever idles; local_scatter costs
num_elems * 1.39ns no matter how few indices, so Pool alone is ~2x too slow
-- that was the old bottleneck, along with a serial ReduceScatter tail).
"""

import dataclasses
import sys

import numpy as np

if "/opt/trn_rl_repo" not in sys.path:
    sys.path.insert(0, "/opt/trn_rl_repo")

import ml_dtypes

import concourse.bacc as bacc
import concourse.bass as bass
import concourse.mybir as mybir
import concourse.tile as tile

F32 = mybir.dt.float32
BF16 = mybir.dt.bfloat16
I16 = mybir.dt.int16
BFNP = ml_dtypes.bfloat16


@dataclasses.dataclass(frozen=True)
class Cfg:
    N: int = 8192           # nodes
    D: int = 128            # features (in == out)
    C: int = 8              # cores
    PERIOD: int = 8         # j-block pattern period
    DMA_PER: int = 5        # first DMA_PER blocks of each period stream from HBM
    PADW: int = 28          # padded per-(j-row) event list width (pool blocks)

    @property
    def R(self):            # output rows per core
        return self.N // self.C

    @property
    def JB(self):           # 128-row j blocks
        return self.N // 128

    @property
    def pool_blocks(self):
        return [b for b in range(self.JB) if b % self.PERIOD >= self.DMA_PER]

    @property
    def dma_runs(self):
        """(start_block, n_blocks) maximal runs of HBM-streamed j blocks."""
        runs = []
        b = 0
        while b < self.JB:
            if b % self.PERIOD < self.DMA_PER:
                n = min(self.DMA_PER - b % self.PERIOD, self.JB - b)
                runs.append((b, n))
                b += n
            else:
                b += 1
        return runs


FULL = Cfg()


def build(cfg: Cfg) -> bass.Bass:
    N, D, R, JB = cfg.N, cfg.D, cfg.R, cfg.JB
    PADW = cfg.PADW
    pool_blocks = cfg.pool_blocks
    NP = len(pool_blocks)
    tloc = {b: t for t, b in enumerate(pool_blocks)}

    nc = bacc.Bacc()

    # x wrapped on host: xw[p, b*128 + c] = x[b*128 + p, c]  (bf16)
    xw = nc.dram_tensor("xw", [128, JB * D], BF16, kind="ExternalInput")
    # scaled adjacency columns of this core: adj[j, i] = dinv_i*dinv_j*A[i,j]
    adj = nc.dram_tensor("adj", [N, R], BF16, kind="ExternalInput")
    wt = nc.dram_tensor("wt", [D, D], BF16, kind="ExternalInput")      # W^T
    lin2 = nc.dram_tensor("lin2", [2, D], BF16, kind="ExternalInput")  # [b_lin; bias]
    srow = nc.dram_tensor("srow", [2, R], BF16, kind="ExternalInput")  # [s; 1]
    # pool-built blocks: per j-row index (dst local, -1 pad) and value lists
    rc = nc.dram_tensor("rc", [128, NP * PADW], I16, kind="ExternalInput")
    rcv = nc.dram_tensor("rcv", [128, NP * PADW], BF16, kind="ExternalInput")
    outT = nc.dram_tensor("outT", [D, R], F32, kind="ExternalOutput")

    with tile.TileContext(nc, num_cores=cfg.C) as tc:
        const_p = tc.alloc_tile_pool(name="const", bufs=1)
        psum_p = tc.alloc_tile_pool(name="psum", bufs=8, space="PSUM")
        dchunk_p = tc.alloc_tile_pool(name="dchunk", bufs=4)
        ptile_p = tc.alloc_tile_pool(name="ptile", bufs=8)
        stage_p = tc.alloc_tile_pool(name="stage", bufs=1)

        # x (stationaries) in 8 chunks, interleaved with adjacency streaming.
        x_sb = const_p.tile([128, JB * D], BF16, name="x_sb")
        XC = 8
        xw_chunk = JB * D // XC

        def load_x(q):
            nc.sync.dma_start(out=x_sb[:, q * xw_chunk:(q + 1) * xw_chunk],
                              in_=xw[:, q * xw_chunk:(q + 1) * xw_chunk])

        # HBM-streamed adjacency: first run split 1+rest so PE starts early.
        runs = []
        for (s, n) in cfg.dma_runs:
            if not runs and n > 1:
                runs += [(s, 1), (s + 1, n - 1)]
            else:
                runs.append((s, n))

        tiles = {}
        rc_sb = const_p.tile([128, NP * PADW], I16, name="rc_sb")
        rcv_sb = const_p.tile([128, NP * PADW], BF16, name="rcv_sb")

        load_x(0)
        xq = 1
        for ri, (s, n) in enumerate(runs):
            if ri >= 2 and xq < XC:
                # keep x one run ahead of the adjacency blocks that use it
                load_x(xq)
                xq += 1
            ch = dchunk_p.tile([128, 5 * 1024], BF16, name="ch")
            nc.sync.dma_start(
                out=ch[:, :n * 1024].rearrange("p (t i) -> p t i", i=1024),
                in_=adj[s * 128:(s + n) * 128, :].rearrange(
                    "(t p) i -> p t i", p=128))
            for t in range(n):
                tiles[s + t] = ch[:, t * 1024:(t + 1) * 1024]
            if ri == 1:
                # Pool's inputs right after the first full adjacency chunk;
                # its first tile isn't consumed until block DMA_PER.
                nc.sync.dma_start(out=rc_sb[:], in_=rc[:])
                nc.sync.dma_start(out=rcv_sb[:], in_=rcv[:])
        while xq < XC:
            load_x(xq)
            xq += 1

        # Pool-built adjacency tiles (emitted up front on the Pool queue;
        # the tile pool's buffer limit paces Pool against PE consumption).
        for b in pool_blocks:
            at = ptile_p.tile([128, 1024], BF16, name="pt")
            nc.gpsimd.local_scatter(
                out_ap=at[:],
                data_ap=rcv_sb[:, tloc[b] * PADW:(tloc[b] + 1) * PADW],
                idxs_ap=rc_sb[:, tloc[b] * PADW:(tloc[b] + 1) * PADW],
                channels=128,
                num_elems=R,
                num_idxs=PADW,
            )
            tiles[b] = at
        # tail-only constants
        wt_sb = const_p.tile([D, D], BF16, name="wt_sb")
        nc.sync.dma_start(out=wt_sb[:], in_=wt[:])
        lin2_sb = const_p.tile([2, D], BF16, name="lin2_sb")
        nc.sync.dma_start(out=lin2_sb[:], in_=lin2[:])
        srow_sb = const_p.tile([2, R], BF16, name="srow_sb")
        nc.sync.dma_start(out=srow_sb[:], in_=srow[:])

        # ---- main: yT[c, i] = sum_b x_blk(b)^T.T @ adj_tile(b) -----------
        ps0 = psum_p.tile([128, 512], F32, name="ps0", bufs=1)
        ps1 = psum_p.tile([128, 512], F32, name="ps1", bufs=1)
        for b in range(JB):
            xb = x_sb[:, b * D:(b + 1) * D]
            first, last = b == 0, b == JB - 1
            nc.tensor.matmul(ps0[:], lhsT=xb, rhs=tiles[b][:, 0:512],
                             start=first, stop=last)
            nc.tensor.matmul(ps1[:], lhsT=xb, rhs=tiles[b][:, 512:1024],
                             start=first, stop=last)

        # ---- tail: outT = wt.T @ yT + lin2.T @ [s; 1] ---------------------
        y_sb = stage_p.tile([128, R], BF16, name="y_sb")
        nc.vector.tensor_copy(y_sb[:, 0:512], ps0[:])
        nc.scalar.copy(y_sb[:, 512:1024], ps1[:])
        psF0 = psum_p.tile([128, 512], F32, name="psF0", bufs=1)
        psF1 = psum_p.tile([128, 512], F32, name="psF1", bufs=1)
        nc.tensor.matmul(psF0[:], lhsT=wt_sb[:], rhs=y_sb[:, 0:512],
                         start=True, stop=False)
        nc.tensor.matmul(psF0[:], lhsT=lin2_sb[:], rhs=srow_sb[:, 0:512],
                         start=False, stop=True)
        nc.tensor.matmul(psF1[:], lhsT=wt_sb[:], rhs=y_sb[:, 512:1024],
                         start=True, stop=False)
        nc.tensor.matmul(psF1[:], lhsT=lin2_sb[:], rhs=srow_sb[:, 512:1024],
                         start=False, stop=True)
        o_sb = stage_p.tile([128, R], F32, name="o_sb")
        nc.vector.tensor_copy(o_sb[:, 0:512], psF0[:])
        nc.sync.dma_start(out=outT[:, 0:512], in_=o_sb[:, 0:512])
        nc.scalar.copy(o_sb[:, 512:1024], psF1[:])
        nc.sync.dma_start(out=outT[:, 512:1024], in_=o_sb[:, 512:1024])

        for p in [stage_p, ptile_p, dchunk_p, psum_p, const_p]:
            p.release()

    return nc


def _bf16(a):
    return np.asarray(a, dtype=np.float32).astype(BFNP)


def make_in_maps(cfg: Cfg, x, edge_index, W, b_lin, bias):
    N, D, C, R, JB = cfg.N, cfg.D, cfg.C, cfg.R, cfg.JB

    x = np.asarray(x, dtype=np.float32)
    W = np.asarray(W, dtype=np.float32)
    b_lin = np.asarray(b_lin, dtype=np.float32)
    bias = np.asarray(bias, dtype=np.float32)
    ei = np.asarray(edge_index).astype(np.int64)

    # symmetrize + dedup (set semantics, matches at[].set)
    key = np.unique(np.concatenate([ei[0] * N + ei[1], ei[1] * N + ei[0]]))
    de = (key // N).astype(np.int64)   # dst (output row)
    sr = (key % N).astype(np.int64)    # src
    deg = np.bincount(de, minlength=N)
    dinv = 1.0 / np.sqrt(deg.astype(np.float64) + 1e-6)
    vals = (dinv[de] * dinv[sr]).astype(np.float32)
    s = (dinv * np.bincount(de, weights=dinv[sr], minlength=N)).astype(np.float32)
    dinv = dinv.astype(np.float32)

    # pool-block event lists: group by (src row, dst core), slot = rank
    core = de // R
    jb = sr // 128
    pool_mask = (jb % cfg.PERIOD) >= cfg.DMA_PER
    pe_sr, pe_de, pe_core = sr[pool_mask], de[pool_mask], core[pool_mask]
    pe_val = vals[pool_mask]
    grp = pe_sr * C + pe_core
    order = np.argsort(grp, kind="stable")
    gs = grp[order]
    cnt = np.bincount(gs, minlength=N * C)
    starts = np.concatenate([[0], np.cumsum(cnt)[:-1]])
    slot = np.arange(gs.size) - np.repeat(starts, cnt)
    padw = int(cnt.max())
    padw = max(4, (padw + 1) // 2 * 2)
    cfg = dataclasses.replace(cfg, PADW=padw)
    pool_blocks = cfg.pool_blocks
    NP = len(pool_blocks)
    tloc_arr = np.full(JB, -1, np.int64)
    for t, b in enumerate(pool_blocks):
        tloc_arr[b] = t

    o_sr, o_de, o_core = pe_sr[order], pe_de[order], pe_core[order]
    o_val = pe_val[order]
    p_row = o_sr % 128
    p_t = tloc_arr[o_sr // 128]
    col = p_t * padw + slot
    rc_all = np.full((C, 128, NP * padw), -1, np.int16)
    rcv_all = np.zeros((C, 128, NP * padw), BFNP)
    rc_all[o_core, p_row, col] = (o_de % R).astype(np.int16)
    rcv_all[o_core, p_row, col] = o_val.astype(BFNP)

    # dense scaled adjacency, bf16; per-core column slices
    A = np.zeros((N, N), BFNP)
    A[sr, de] = vals.astype(BFNP)

    xw = np.ascontiguousarray(
        _bf16(x).reshape(JB, 128, D).transpose(1, 0, 2).reshape(128, JB * D))
    wt = np.ascontiguousarray(_bf16(W.T))
    lin2 = np.ascontiguousarray(_bf16(np.stack([b_lin, bias])))

    in_maps = []
    for k in range(C):
        sk = np.empty((2, R), np.float32)
        sk[0] = s[k * R:(k + 1) * R]
        sk[1] = 1.0
        in_maps.append({
            "xw": xw,
            "adj": np.ascontiguousarray(A[:, k * R:(k + 1) * R]),
            "wt": wt,
            "lin2": lin2,
            "srow": sk.astype(BFNP),
            "rc": rc_all[k],
            "rcv": rcv_all[k],
        })
    return cfg, in_maps


def kernel(x, edge_index, W, b_lin, bias, *, trace=False, cfg: Cfg = FULL):
    from concourse.bass_utils import run_bass_kernel_spmd

    if trace:
        _install_ntff_hook()
    cfg, in_maps = make_in_maps(cfg, x, edge_index, W, b_lin, bias)
    nc = build(cfg)
    nc.finalize()
    res = run_bass_kernel_spmd(nc, in_maps, core_ids=list(range(cfg.C)),
                               trace=trace)
    full = np.concatenate(
        [np.asarray(r["outT"]).T for r in res.results], axis=0)
    kernel.last_results = res
    return np.ascontiguousarray(full).astype(np.float32)


kernel.last_results = None


def _install_ntff_hook():
    """Provide antenv.axon_hooks (missing on this image) so that
    run_bass_kernel_spmd(trace=True) can capture NTFF profiles via the
    axon ctypes hook from trn_agent_boot."""
    import sys as _sys
    import types

    try:
        import antenv.axon_hooks  # noqa: F401
        return True
    except ImportError:
        pass
    try:
        import antenv
        from trn_agent_boot.trn_boot import _ntff_profile_via_ctypes

        hook = _ntff_profile_via_ctypes("/opt/axon/libaxon_pjrt.so")
        mod = types.ModuleType("antenv.axon_hooks")
        mod.get_axon_ntff_profile_hook = lambda: hook
        mod.set_axon_ntff_profile_hook = lambda h: None
        _sys.modules["antenv.axon_hooks"] = mod
        antenv.axon_hooks = mod
        return hook is not None
    except Exception as e:  # profiling is best-effort
        print(f"ntff hook install failed: {e}", file=sys.stderr)
        return False
